# revision 1
# baseline (speedup 1.0000x reference)
"""Trainium2 Bass kernel for nn_ContrastiveMSELoss (8192x8192 cos-sim contrastive + MSE).

Sharding: 8 NeuronCores, users row-sharded 1024/core, full recipe table per core.

The loss decomposes so the 8192x8192 ratings matrix is never materialized:
    rowR[i]  = 0.1*M + sum_{final scatter cells in row i}(v - 0.1)
    S1       = 0.1*T + sum_pairs (v-0.1)*cos[u,i],  T = (sum_i u_i/|u_i|) . (sum_j r_j/|r_j|)
    S2       = sum_i rowR[i] * log(rowsum_exp[i])
    S3       = sum_i rowR[i] * log(colsum_exp[i])    (col_sum indexed by i: torch n==m quirk)
    loss     = 0.5*(S2 + S3 - 2*S1)/(2*N) + 0.5*mean((ratings-cos_sim)^2)

Per core: normalize R in a per-1024-column pipelined chain (square/reduce/ln/exp/mul ->
PE transpose -> bf16 [64, 8192]); cos tiles via PE matmul with 1/|u| folded into the ACT
exp's per-partition scale; row sums via a DVE tensor_scalar+accum pass over the bf16 exp
tiles; column sums via ones-matmul PSUM accumulation (two concurrent col-tiled matmuls);
per-block colsum partials DMA'd to DRAM progressively and ReduceScatter'd across cores;
scattered-pair cos via dma_gather of pre-normalized rows + DVE dots, scheduled in the
collective's shadow. Host does index prep (dedup last-write-wins, bincount, sharding)
and sums 8x5 partial scalars.
"""

import sys

sys.path.insert(0, "/opt/trn_rl_repo")

import numpy as np

import concourse.bass as bass
import concourse.bacc as bacc
import concourse.tile as tile
from concourse import mybir
from concourse.bass_utils import run_bass_kernel_spmd
from concourse.masks import make_identity

f32 = mybir.dt.float32
bf16 = mybir.dt.bfloat16
i16 = mybir.dt.int16
AF = mybir.ActivationFunctionType
OP = mybir.AluOpType
AX = mybir.AxisListType

NCORES = 8
N = 8192          # users
M = 8192          # recipes
D = 64
B = 65536
S = N // NCORES   # slab rows per core (1024)
RT = S // 128     # row tiles per slab (8)
NG = 8            # column groups of 1024
ALPHA = 0.5
FILL = 0.1
GATHER_CHUNK = 512  # descriptor-ring capacity limits idxs per dma_gather


def build_nc(K):
    """Build the SPMD Bass program. K = pairs per partition (128*K pair slots/core)."""
    nc = bacc.Bacc(num_devices=NCORES)

    u_slab = nc.declare_dram_parameter("u_slab", [S, D], f32, isOutput=False)
    r_full = nc.declare_dram_parameter("r_full", [M, D], f32, isOutput=False)
    row_r_slab = nc.declare_dram_parameter("row_r_slab", [S], f32, isOutput=False)
    pair_u = nc.declare_dram_parameter("pair_u", [128, 8 * K], i16, isOutput=False)
    pair_i = nc.declare_dram_parameter("pair_i", [128, 8 * K], i16, isOutput=False)
    pair_w = nc.declare_dram_parameter("pair_w", [128 * K], f32, isOutput=False)
    mse_ab = nc.declare_dram_parameter("mse_ab", [2 * (B // NCORES)], f32, isOutput=False)
    out_d = nc.declare_dram_parameter("out", [1, 8], f32, isOutput=True)

    NP = 128 * K

    with tile.TileContext(nc) as tc:
        with tc.tile_pool(name="sb", bufs=1) as sb, \
             tc.tile_pool(name="dram", bufs=1, space="DRAM") as dpool:
            # ---- constants ----
            ident = sb.tile([128, 128], f32)
            make_identity(nc, ident[:])
            ones_bf = sb.tile([128, 1], bf16)
            nc.vector.memset(ones_bf[:], 1.0)
            ones_f = sb.tile([128, 1], f32)
            nc.vector.memset(ones_f[:], 1.0)

            # ---- input loads ----
            u_raw = sb.tile([128, RT, D], f32)   # user r*128+p -> [p, r, :]
            nc.sync.dma_start(out=u_raw[:], in_=u_slab[:].rearrange("(r p) d -> p r d", p=128))
            r_raw = sb.tile([128, 64, D], f32)   # recipe n*128+p -> [p, n, :]
            nc.sync.dma_start(out=r_raw[:], in_=r_full[:].rearrange("(n p) d -> p n d", p=128))
            pu = sb.tile([128, NP // 16], i16)
            nc.sync.dma_start(out=pu[:], in_=pair_u[:])
            pi = sb.tile([128, NP // 16], i16)
            nc.sync.dma_start(out=pi[:], in_=pair_i[:])

            un_dram = dpool.tile([S, D], f32)
            rn_dram = dpool.tile([M, D], f32)
            cc_in = dpool.tile([M], f32)
            cc_out = dpool.tile([S], f32)

            with tc.tile_pool(name="psM", bufs=1, space="PSUM") as psM:
                # ---- U chain: invu + normalized copy + transpose ----
                usq = sb.tile([128, RT, D], f32)
                nc.vector.tensor_tensor(out=usq[:], in0=u_raw[:], in1=u_raw[:], op=OP.mult)
                ssq_u = sb.tile([128, RT], f32)
                nc.vector.tensor_reduce(out=ssq_u[:], in_=usq[:], axis=AX.X, op=OP.add)
                lssq_u = sb.tile([128, RT], f32)
                nc.scalar.activation(out=lssq_u[:], in_=ssq_u[:], func=AF.Ln)
                invu = sb.tile([128, RT], f32)
                nc.scalar.activation(out=invu[:], in_=lssq_u[:], func=AF.Exp, scale=-0.5)
                un = sb.tile([128, RT, D], f32)
                nc.vector.tensor_tensor(
                    out=un[:], in0=u_raw[:],
                    in1=invu[:, :, None].to_broadcast([128, RT, D]), op=OP.mult)
                nc.sync.dma_start(out=un_dram[:].rearrange("(r p) d -> p r d", p=128), in_=un[:])

                UT = sb.tile([64, S], bf16)
                ptu = psM.tile([64, 1024], f32, tag="tr", bufs=1)
                for r in range(RT):
                    nc.tensor.transpose(
                        out=ptu[:, r * 128:(r + 1) * 128], in_=u_raw[:, r, :], identity=ident[:])
                nc.vector.tensor_copy(out=UT[:], in_=ptu[:])

                # pair U gathers can start as soon as un_dram lands
                ug = sb.tile([128, K, D], f32)
                rg = sb.tile([128, K, D], f32)
                for off in range(0, NP, GATHER_CHUNK):
                    n = min(GATHER_CHUNK, NP - off)
                    nc.gpsimd.dma_gather(
                        ug[:, off // 128:(off + n) // 128, :], un_dram[:],
                        pu[:, off // 16:(off + n) // 16], n, n, D)

                # ---- R normalize pipeline (per column group g of 1024) ----
                RT_sb = sb.tile([64, M], bf16)
                sr_parts = sb.tile([64, NG], f32)
                ssq_r = sb.tile([128, 64], f32)
                invr = sb.tile([128, 64], f32)
                rhat = sb.tile([128, 64, D], f32)

                def phase_a(g):
                    gs = slice(g * 8, (g + 1) * 8)
                    rsq = sb.tile([128, 8, D], f32, tag="rsq", bufs=2)
                    nc.vector.tensor_tensor(
                        out=rsq[:], in0=r_raw[:, gs, :], in1=r_raw[:, gs, :], op=OP.mult)
                    nc.vector.tensor_reduce(
                        out=ssq_r[:, gs], in_=rsq[:], axis=AX.X, op=OP.add)
                    lss = sb.tile([128, 8], f32, tag="lss", bufs=2)
                    nc.scalar.activation(out=lss[:], in_=ssq_r[:, gs], func=AF.Ln)
                    nc.scalar.activation(out=invr[:, gs], in_=lss[:], func=AF.Exp, scale=-0.5)
                    nc.vector.tensor_tensor(
                        out=rhat[:, gs, :], in0=r_raw[:, gs, :],
                        in1=invr[:, gs][:, :, None].to_broadcast([128, 8, D]), op=OP.mult)
                    # write normalized rows to DRAM for the pair gathers
                    nc.sync.dma_start(
                        out=rn_dram[g * 1024:(g + 1) * 1024, :].rearrange("(n p) d -> p n d", p=128),
                        in_=rhat[:, gs, :])
                    ptr = psM.tile([64, 1024], f32, tag="tr", bufs=1)
                    for t in range(8):
                        nc.tensor.transpose(
                            out=ptr[:, t * 128:(t + 1) * 128], in_=rhat[:, g * 8 + t, :],
                            identity=ident[:])
                    nc.vector.tensor_scalar(
                        out=RT_sb[:, g * 1024:(g + 1) * 1024], in0=ptr[:],
                        scalar1=1.0, scalar2=None, op0=OP.mult, op1=OP.add,
                        accum_out=sr_parts[:, g:g + 1])

                # ---- main loop state ----
                rs_acc = sb.tile([128, RT * NG], f32)

                def phase_b(g):
                    cs_g = psM.tile([128, 512], f32, tag="cs", bufs=2, name=f"cs{g}")
                    for r in range(RT):
                        pg = psM.tile([128, 1024], f32, tag="cos", bufs=2)
                        for jj in range(2):
                            j = g * 2 + jj
                            nc.tensor.matmul(
                                out=pg[:, jj * 512:(jj + 1) * 512],
                                lhsT=UT[:, r * 128:(r + 1) * 128],
                                rhs=RT_sb[:, j * 512:(j + 1) * 512],
                                start=True, stop=True)
                        ex = sb.tile([128, 1024], bf16, tag="exp", bufs=4)
                        nc.scalar.activation(
                            out=ex[:], in_=pg[:], func=AF.Exp, scale=invu[:, r:r + 1])
                        tspo = sb.tile([128, 1024], bf16, tag="tsp", bufs=2)
                        nc.vector.tensor_scalar(
                            out=tspo[:], in0=ex[:], scalar1=1.0, scalar2=None,
                            op0=OP.mult, op1=OP.add,
                            accum_out=rs_acc[:, r * NG + g:r * NG + g + 1])
                        for jj in range(2):
                            nc.tensor.matmul(
                                out=cs_g[32 * jj:32 * jj + 1, :],
                                lhsT=ones_bf[:, 0:1],
                                rhs=ex[:, jj * 512:(jj + 1) * 512],
                                start=(r == 0), stop=(r == RT - 1),
                                tile_position=(0, 32 * jj),
                                skip_group_check=True)
                    # colsum partials for blocks 2g, 2g+1 -> DRAM (progressive)
                    csb = sb.tile([128, 2, 512], f32, tag="csb", bufs=2, name=f"csb{g}")
                    nc.vector.tensor_copy(out=csb[0:1, 0, :], in_=cs_g[0:1, :])
                    nc.vector.tensor_copy(out=csb[32:33, 1, :], in_=cs_g[32:33, :])
                    nc.sync.dma_start(out=cc_in[(2 * g) * 512:(2 * g + 1) * 512], in_=csb[0:1, 0, :])
                    nc.sync.dma_start(out=cc_in[(2 * g + 1) * 512:(2 * g + 2) * 512], in_=csb[32:33, 1, :])

                for g in range(NG):
                    phase_a(g)
                # rn_dram writes all issued; R-row gathers are ordered after them
                for off in range(0, NP, GATHER_CHUNK):
                    n = min(GATHER_CHUNK, NP - off)
                    nc.gpsimd.dma_gather(
                        rg[:, off // 128:(off + n) // 128, :], rn_dram[:],
                        pi[:, off // 16:(off + n) // 16], n, n, D)
                for g in range(NG):
                    phase_b(g)

            # =============== collective: ReduceScatter column sums ===============
            nc.gpsimd.collective_compute(
                "ReduceScatter", OP.add,
                replica_groups=[list(range(NCORES))],
                ins=[cc_in[:].opt()], outs=[cc_out[:].opt()])

            # =============== tail (overlaps the collective) ===============
            with tc.tile_pool(name="psT", bufs=1, space="PSUM") as psT:
                # T partial: sum_{p,r} invu * (u_raw . sR)
                sr_f = sb.tile([64, 1], f32)
                nc.vector.tensor_reduce(out=sr_f[:], in_=sr_parts[:], axis=AX.X, op=OP.add)
                sr_bf = sb.tile([64, 1], bf16)
                nc.vector.tensor_copy(out=sr_bf[:], in_=sr_f[:])
                psTT = psT.tile([128, RT], f32)
                for r in range(RT):
                    nc.tensor.matmul(
                        out=psTT[:, r:r + 1], lhsT=UT[:, r * 128:(r + 1) * 128],
                        rhs=sr_bf[:], start=True, stop=True)
                tdot = sb.tile([128, RT], f32)
                nc.vector.tensor_copy(out=tdot[:], in_=psTT[:])
                tw = sb.tile([128, RT], f32)
                nc.vector.tensor_tensor(out=tw[:], in0=tdot[:], in1=invu[:], op=OP.mult)
                t_acc = sb.tile([128, 1], f32)
                nc.vector.tensor_reduce(out=t_acc[:], in_=tw[:], axis=AX.X, op=OP.add)

                # pair term: cos = un[u] . rn[i] (rows pre-normalized)
                prod = sb.tile([128, K, D], f32)
                nc.vector.tensor_tensor(out=prod[:], in0=ug[:], in1=rg[:], op=OP.mult)
                cosg = sb.tile([128, K], f32)
                nc.vector.tensor_reduce(out=cosg[:], in_=prod[:], axis=AX.X, op=OP.add)
                pw = sb.tile([128, K], f32)
                nc.sync.dma_start(out=pw[:], in_=pair_w[:].rearrange("(c p) -> p c", p=128))
                cosgw = sb.tile([128, K], f32)
                nc.vector.tensor_tensor(out=cosgw[:], in0=cosg[:], in1=pw[:], op=OP.mult)
                w_acc = sb.tile([128, 1], f32)
                nc.vector.tensor_reduce(out=w_acc[:], in_=cosgw[:], axis=AX.X, op=OP.add)

                # S2: sum rowR_slab * ln(rowsum)
                rs_r = sb.tile([128, RT], f32)
                nc.vector.tensor_reduce(
                    out=rs_r[:], in_=rs_acc[:].rearrange("p (r g) -> p r g", g=NG),
                    axis=AX.X, op=OP.add)
                lrs = sb.tile([128, RT], f32)
                nc.scalar.activation(out=lrs[:], in_=rs_r[:], func=AF.Ln)
                rowr_sb = sb.tile([128, RT], f32)
                nc.sync.dma_start(out=rowr_sb[:], in_=row_r_slab[:].rearrange("(r p) -> p r", p=128))
                s2w = sb.tile([128, RT], f32)
                nc.vector.tensor_tensor(out=s2w[:], in0=lrs[:], in1=rowr_sb[:], op=OP.mult)
                s2_acc = sb.tile([128, 1], f32)
                nc.vector.tensor_reduce(out=s2_acc[:], in_=s2w[:], axis=AX.X, op=OP.add)

                # MSE
                mab = sb.tile([128, 128], f32)
                nc.sync.dma_start(out=mab[:], in_=mse_ab[:].rearrange("(p k) -> p k", p=128))
                md = sb.tile([128, 64], f32)
                nc.vector.tensor_tensor(out=md[:], in0=mab[:, 0:64], in1=mab[:, 64:128], op=OP.subtract)
                msq = sb.tile([128, 64], f32)
                nc.vector.tensor_tensor(out=msq[:], in0=md[:], in1=md[:], op=OP.mult)
                m_acc = sb.tile([128, 1], f32)
                nc.vector.tensor_reduce(out=m_acc[:], in_=msq[:], axis=AX.X, op=OP.add)

                # S3 (after ReduceScatter lands): sum rowR_slab * ln(colsum_slab)
                lcs_in = sb.tile([128, RT], f32)
                nc.sync.dma_start(out=lcs_in[:], in_=cc_out[:].rearrange("(r p) -> p r", p=128))
                lcs = sb.tile([128, RT], f32)
                nc.scalar.activation(out=lcs[:], in_=lcs_in[:], func=AF.Ln)
                s3w = sb.tile([128, RT], f32)
                nc.vector.tensor_tensor(out=s3w[:], in0=lcs[:], in1=rowr_sb[:], op=OP.mult)
                s3_acc = sb.tile([128, 1], f32)
                nc.vector.tensor_reduce(out=s3_acc[:], in_=s3w[:], axis=AX.X, op=OP.add)

                # partition-reduce the five partials via ones-matmul
                combo = sb.tile([128, 5], f32)
                nc.vector.tensor_copy(out=combo[:, 0:1], in_=s2_acc[:])
                nc.vector.tensor_copy(out=combo[:, 1:2], in_=s3_acc[:])
                nc.vector.tensor_copy(out=combo[:, 2:3], in_=t_acc[:])
                nc.vector.tensor_copy(out=combo[:, 3:4], in_=w_acc[:])
                nc.vector.tensor_copy(out=combo[:, 4:5], in_=m_acc[:])
                po = psT.tile([1, 5], f32)
                nc.tensor.matmul(out=po[:], lhsT=ones_f[:, 0:1], rhs=combo[:], start=True, stop=True)
                out_sb = sb.tile([1, 8], f32)
                nc.vector.memset(out_sb[:], 0.0)
                nc.vector.tensor_copy(out=out_sb[:, 0:5], in_=po[:])
                nc.sync.dma_start(out=out_d[:], in_=out_sb[:])
    nc.finalize()
    return nc


def _host_prep(inputs):
    """Dedup scatter (last write wins), shard pairs by row slab, build per-core arrays."""
    U = np.ascontiguousarray(np.asarray(inputs["user_embeddings"], dtype=np.float32))
    R = np.ascontiguousarray(np.asarray(inputs["recipe_embeddings"], dtype=np.float32))
    rat = np.asarray(inputs["ratings_scaled"], dtype=np.float32)
    css = np.asarray(inputs["cos_similarities_scaled"], dtype=np.float32)
    u = np.asarray(inputs["u_idx"]).astype(np.int64)
    i = np.asarray(inputs["i_idx"]).astype(np.int64)

    cell = u * M + i
    _, idx_rev = np.unique(cell[::-1], return_index=True)
    keep = (B - 1 - idx_rev)  # last occurrences, sorted by cell (=> sorted by u)
    uu = u[keep].astype(np.int32)
    ii = i[keep].astype(np.int32)
    ww = (rat[keep] - FILL).astype(np.float32)

    delta = np.bincount(uu, weights=ww.astype(np.float64), minlength=N)
    row_r = (FILL * M + delta).astype(np.float32)

    core_of = uu // S
    counts = np.bincount(core_of, minlength=NCORES)
    K = int(np.ceil(counts.max() / 128))
    cap = 128 * K

    in_maps = []
    bs = B // NCORES
    for c in range(NCORES):
        sel = core_of == c
        n_c = int(sel.sum())
        pu = np.zeros(cap, dtype=np.int16)
        pi = np.zeros(cap, dtype=np.int16)
        pw = np.zeros(cap, dtype=np.float32)
        pu[:n_c] = uu[sel] - c * S
        pi[:n_c] = ii[sel]
        pw[:n_c] = ww[sel]
        # dma_gather idx layout: [128, cap//16], row p = idx[s*16 + p%16], replicated 8x
        pu_dev = np.ascontiguousarray(np.tile(pu.reshape(cap // 16, 16).T, (8, 1)).astype(np.int16))
        pi_dev = np.ascontiguousarray(np.tile(pi.reshape(cap // 16, 16).T, (8, 1)).astype(np.int16))
        in_maps.append({
            "u_slab": np.ascontiguousarray(U[c * S:(c + 1) * S]),
            "r_full": R,
            "row_r_slab": np.ascontiguousarray(row_r[c * S:(c + 1) * S]),
            "pair_u": pu_dev,
            "pair_i": pi_dev,
            "pair_w": pw,
            "mse_ab": np.concatenate([
                rat[c * bs:(c + 1) * bs].reshape(128, 64),
                css[c * bs:(c + 1) * bs].reshape(128, 64)], axis=1).ravel(),
        })
    return in_maps, K


def kernel(user_embeddings, recipe_embeddings, ratings_scaled, cos_similarities_scaled,
           u_idx, i_idx, _trace=False):
    inputs = {
        "user_embeddings": user_embeddings,
        "recipe_embeddings": recipe_embeddings,
        "ratings_scaled": ratings_scaled,
        "cos_similarities_scaled": cos_similarities_scaled,
        "u_idx": u_idx,
        "i_idx": i_idx,
    }
    in_maps, K = _host_prep(inputs)
    nc = build_nc(K)
    res = run_bass_kernel_spmd(nc, in_maps, core_ids=list(range(NCORES)), trace=_trace)
    outs = np.stack([res.results[c]["out"][0] for c in range(NCORES)])  # [8, 8]
    o = outs.astype(np.float64)
    S2 = o[:, 0].sum()
    S3 = o[:, 1].sum()
    T = o[:, 2].sum()
    PAIR = o[:, 3].sum()
    MSE_SUM = o[:, 4].sum()
    contrastive = (S2 + S3 - 2.0 * (FILL * T + PAIR)) / (2.0 * N)
    loss = ALPHA * contrastive + (1.0 - ALPHA) * (MSE_SUM / B)
    if _trace:
        kernel._last_results = res
    return np.float32(loss)



# revision 3
# speedup vs baseline: 2.2191x; 2.2191x over previous
"""Trainium2 Bass kernel for nn_ContrastiveMSELoss (8192x8192 cos-sim contrastive + MSE).

Sharding: 8 NeuronCores, users row-sharded 1024/core, full recipe table per core.

The loss decomposes so the 8192x8192 ratings matrix is never materialized:
    rowR[i]  = 0.1*M + sum_{final scatter cells in row i}(v - 0.1)
    S1       = sum_pairs (v-0.1)*cos[u,i]
    T        = sum_ij cos_ij = (sum_i u_i/|u_i|) . (sum_j r_j/|r_j|)
    S2       = sum_i rowR[i] * log(rowsum_exp[i])
    S3       = sum_i rowR[i] * log(colsum_exp[i])    (col_sum indexed by i: torch n==m quirk)
    loss     = 0.5*(S2 + S3 - 2*(0.1*T + S1))/(2*N) + 0.5*mean((ratings-cos_sim)^2)

Per core (ACT-exp-bound design, ~64us of exp is the floor):
  - PE: 128 bf16 cos matmuls [64x128]^T@[64x512]; 72 transposes; 16 selector
    matmuls that partition-reduce the per-group column accumulators into one
    [8,1024] PSUM tile.
  - ACT: one fused op per cos tile: exp(scale*psum) -> bf16 ex tile PLUS
    accum_out row-sum partial (rowsum comes for free). Norms via Ln+Exp(-0.5x)
    so every ACT op uses the single natural_log_exp table (no table thrash).
  - DVE: pairwise-tree adds of the 8 ex tiles per column group (colsum
    accumulate), PSUM->SBUF transpose copies, small reductions.
  - GpSimd: squares for norms, bf16 casts, r-normalize muls, pair-term
    elementwise products (otherwise idle: no gathers, no collective).
  - No collective at all: each core ships its [8,1024] colsum partials; host
    sums 8x8192 floats, takes log, dots with rowR (O(N) host work, same scale
    as the host bincount for rowR).
  - Pair term: host pre-gathers the (deduped, u-sharded) pair embedding rows
    as bf16; device computes dots + norms + weighted sum. No dma_gather.
"""

import sys

sys.path.insert(0, "/opt/trn_rl_repo")

import numpy as np
import ml_dtypes

import concourse.bass as bass
import concourse.bacc as bacc
import concourse.tile as tile
from concourse import mybir
from concourse.bass_utils import run_bass_kernel_spmd
from concourse.masks import make_identity

f32 = mybir.dt.float32
bf16 = mybir.dt.bfloat16
AF = mybir.ActivationFunctionType
OP = mybir.AluOpType
AX = mybir.AxisListType

NCORES = 8
N = 8192          # users
M = 8192          # recipes
D = 64
B = 65536
S = N // NCORES   # slab rows per core (1024)
NG = 8            # column groups of 1024
ALPHA = 0.5
FILL = 0.1


def build_nc(K):
    """SPMD Bass program. K = pair slots per partition (128*K pairs/core)."""
    nc = bacc.Bacc(num_devices=NCORES)

    u_d = nc.declare_dram_parameter("u_d", [128, 512], f32, isOutput=False)
    r_d = nc.declare_dram_parameter("r_d", [128, 4096], f32, isOutput=False)
    rowr_d = nc.declare_dram_parameter("rowr_d", [128, 8], f32, isOutput=False)
    up_d = nc.declare_dram_parameter("up_d", [128, K * 64], bf16, isOutput=False)
    rp_d = nc.declare_dram_parameter("rp_d", [128, K * 64], bf16, isOutput=False)
    pw_d = nc.declare_dram_parameter("pw_d", [128, K], f32, isOutput=False)
    mse_d = nc.declare_dram_parameter("mse_d", [128, 128], f32, isOutput=False)
    out_d = nc.declare_dram_parameter("out", [1, 8], f32, isOutput=True)
    cs_d = nc.declare_dram_parameter("cs", [8, 1024], f32, isOutput=True)

    with tile.TileContext(nc) as tc:
        with tc.tile_pool(name="sb", bufs=1) as sb:
            # ---- constants ----
            ident = sb.tile([128, 128], bf16)
            make_identity(nc, ident[:])
            sel = sb.tile([128, 8, 8], bf16)
            nc.vector.memset(sel[:], 0.0)
            for g in range(NG):
                nc.vector.memset(sel[:, g, g:g + 1], 1.0)
            ones_f = sb.tile([128, 1], f32)
            nc.vector.memset(ones_f[:], 1.0)

            # ---- input loads (emission order = priority on the sync queue) ----
            r_sb = sb.tile([128, 64, 64], f32)   # recipe p*64+n -> [p, n, :]
            for g in range(NG):
                nc.sync.dma_start(
                    out=r_sb[:, g * 8:(g + 1) * 8, :],
                    in_=r_d[:, g * 512:(g + 1) * 512].rearrange("p (n d) -> p n d", d=D))
            u_sb = sb.tile([128, 8, 64], f32)    # user q*8+r -> [q, r, :]
            nc.sync.dma_start(out=u_sb[:], in_=u_d[:].rearrange("p (n d) -> p n d", d=D))
            rowr_sb = sb.tile([128, 8], f32)
            nc.sync.dma_start(out=rowr_sb[:], in_=rowr_d[:])
            mse_sb = sb.tile([128, 128], f32)
            nc.sync.dma_start(out=mse_sb[:], in_=mse_d[:])
            ug = sb.tile([128, K, 64], bf16)     # pair p*K+k: raw user rows
            nc.sync.dma_start(out=ug[:], in_=up_d[:].rearrange("p (k d) -> p k d", d=D))
            rg = sb.tile([128, K, 64], bf16)
            nc.sync.dma_start(out=rg[:], in_=rp_d[:].rearrange("p (k d) -> p k d", d=D))
            pw_sb = sb.tile([128, K], f32)
            nc.sync.dma_start(out=pw_sb[:], in_=pw_d[:])

            # ---- norms: squares on gpsimd, reduces on DVE, Ln/Exp on ACT ----
            u_bf = sb.tile([128, 8, 64], bf16)
            nc.gpsimd.tensor_copy(out=u_bf[:], in_=u_sb[:])
            usq = sb.tile([128, 8, 64], f32)
            nc.gpsimd.tensor_tensor(out=usq[:], in0=u_sb[:], in1=u_sb[:], op=OP.mult)
            ssq = sb.tile([128, 72], f32)        # cols 0:8 = |u|^2, 8:72 = |r|^2
            nc.vector.tensor_reduce(out=ssq[:, 0:8], in_=usq[:], axis=AX.X, op=OP.add)
            for g in range(NG):
                gs = slice(g * 8, (g + 1) * 8)
                rsq = sb.tile([128, 8, 64], f32, tag="rsq", bufs=2)
                nc.gpsimd.tensor_tensor(out=rsq[:], in0=r_sb[:, gs, :], in1=r_sb[:, gs, :], op=OP.mult)
                nc.vector.tensor_reduce(
                    out=ssq[:, 8 + g * 8:16 + g * 8], in_=rsq[:], axis=AX.X, op=OP.add)
            lnssq = sb.tile([128, 72], f32)
            nc.scalar.activation(out=lnssq[:], in_=ssq[:], func=AF.Ln)
            inv_all = sb.tile([128, 72], f32)    # 1/sqrt(ssq): invu cols 0:8, invr 8:72
            nc.scalar.activation(out=inv_all[:], in_=lnssq[:], func=AF.Exp, scale=-0.5)

            with tc.tile_pool(name="ps", bufs=1, space="PSUM") as ps:
                ps_cs = ps.tile([8, 1024], f32, tag="cs")

                # ---- U transpose -> UT [64 dims, 1024 users], user = q*8+r at col r*128+q
                ptu = ps.tile([64, 1024], bf16, tag="tr", bufs=2)
                for r in range(8):
                    nc.tensor.transpose(
                        out=ptu[:, r * 128:(r + 1) * 128], in_=u_bf[:, r, :], identity=ident[:])
                UT = sb.tile([64, 1024], bf16)
                nc.vector.tensor_copy(out=UT[:], in_=ptu[:])

                RT = sb.tile([64, 8192], bf16)   # normalized recipes, dim-major
                sr_parts = sb.tile([64, 8], f32)
                rs_acc = sb.tile([128, 64], f32)
                colacc = sb.tile([128, 8, 1024], bf16)

                def phase_a(g):
                    gs = slice(g * 8, (g + 1) * 8)
                    rhat = sb.tile([128, 8, 64], bf16, tag="rhat", bufs=2)
                    nc.gpsimd.tensor_tensor(
                        out=rhat[:], in0=r_sb[:, gs, :],
                        in1=inv_all[:, 8 + g * 8:16 + g * 8][:, :, None].to_broadcast([128, 8, 64]),
                        op=OP.mult)
                    ptr = ps.tile([64, 1024], bf16, tag="tr", bufs=2)
                    for t in range(8):
                        nc.tensor.transpose(
                            out=ptr[:, t * 128:(t + 1) * 128], in_=rhat[:, t, :],
                            identity=ident[:])
                    nc.vector.tensor_scalar(
                        out=RT[:, g * 1024:(g + 1) * 1024], in0=ptr[:],
                        scalar1=1.0, scalar2=None, op0=OP.mult, op1=OP.add,
                        accum_out=sr_parts[:, g:g + 1])

                def phase_b(g):
                    ex = sb.tile([128, 8, 1024], bf16, tag="ex", bufs=2)
                    for r in range(8):
                        pg = ps.tile([128, 1024], f32, tag="cos", bufs=2)
                        for jj in range(2):
                            nc.tensor.matmul(
                                out=pg[:, jj * 512:(jj + 1) * 512],
                                lhsT=UT[:, r * 128:(r + 1) * 128],
                                rhs=RT[:, g * 1024 + jj * 512:g * 1024 + (jj + 1) * 512],
                                start=True, stop=True)
                        nc.scalar.activation(
                            out=ex[:, r, :], in_=pg[:], func=AF.Exp,
                            scale=inv_all[:, r:r + 1],
                            accum_out=rs_acc[:, r * 8 + g:r * 8 + g + 1])
                    return ex

                def col_tree(g, ex):
                    # colacc[:, g, :] = sum_r ex[:, r, :] via pairwise tree (bf16)
                    lv1 = []
                    for i in range(4):
                        t = sb.tile([128, 1024], bf16, tag="ta", bufs=4)
                        nc.vector.tensor_tensor(
                            out=t[:], in0=ex[:, 2 * i, :], in1=ex[:, 2 * i + 1, :], op=OP.add)
                        lv1.append(t)
                    lv2 = []
                    for i in range(2):
                        t = sb.tile([128, 1024], bf16, tag="tb", bufs=2)
                        nc.vector.tensor_tensor(
                            out=t[:], in0=lv1[2 * i][:], in1=lv1[2 * i + 1][:], op=OP.add)
                        lv2.append(t)
                    nc.vector.tensor_tensor(
                        out=colacc[:, g, :], in0=lv2[0][:], in1=lv2[1][:], op=OP.add)

                def sel_mm(g):
                    for jj in range(2):
                        nc.tensor.matmul(
                            out=ps_cs[:, jj * 512:(jj + 1) * 512],
                            lhsT=sel[:, g, :],
                            rhs=colacc[:, g, jj * 512:(jj + 1) * 512],
                            start=(g == 0), stop=(g == NG - 1),
                            skip_group_check=True)

                # pair-term tiles
                prod = sb.tile([128, K, 64], bf16)
                usqp = sb.tile([128, K, 64], bf16)
                rsqp = sb.tile([128, K, 64], bf16)
                dots = sb.tile([128, K], f32)
                uu = sb.tile([128, K], f32)
                rr = sb.tile([128, K], f32)

                phase_a(0)
                phase_a(1)
                for g in range(NG):
                    if g + 2 < NG:
                        phase_a(g + 2)
                    if g == 5:
                        # gpsimd stream: all rhat muls emitted; pairs fill the rest
                        nc.gpsimd.tensor_tensor(out=prod[:], in0=ug[:], in1=rg[:], op=OP.mult)
                        nc.gpsimd.tensor_tensor(out=usqp[:], in0=ug[:], in1=ug[:], op=OP.mult)
                        nc.gpsimd.tensor_tensor(out=rsqp[:], in0=rg[:], in1=rg[:], op=OP.mult)
                    if g == 7:
                        nc.vector.tensor_reduce(out=dots[:], in_=prod[:], axis=AX.X, op=OP.add)
                        nc.vector.tensor_reduce(out=uu[:], in_=usqp[:], axis=AX.X, op=OP.add)
                        nc.vector.tensor_reduce(out=rr[:], in_=rsqp[:], axis=AX.X, op=OP.add)
                    ex = phase_b(g)
                    col_tree(g, ex)
                    if g >= 1:
                        sel_mm(g - 1)
                    if g == 2:
                        md = sb.tile([128, 64], f32)
                        nc.vector.tensor_tensor(
                            out=md[:], in0=mse_sb[:, 0:64], in1=mse_sb[:, 64:128], op=OP.subtract)
                        msq = sb.tile([128, 64], f32)
                        nc.vector.tensor_tensor(out=msq[:], in0=md[:], in1=md[:], op=OP.mult)
                        m_acc = sb.tile([128, 1], f32)
                        nc.vector.tensor_reduce(out=m_acc[:], in_=msq[:], axis=AX.X, op=OP.add)
                sel_mm(NG - 1)
                cs_sb = sb.tile([8, 1024], f32)
                nc.vector.tensor_copy(out=cs_sb[:], in_=ps_cs[:])
                nc.sync.dma_start(out=cs_d[:], in_=cs_sb[:])

            # =============== tail ===============
            with tc.tile_pool(name="psT", bufs=1, space="PSUM") as psT:
                # T partial: sum_q,r invu * (u . sumRhat)
                sr_f = sb.tile([64, 1], f32)
                nc.vector.tensor_reduce(out=sr_f[:], in_=sr_parts[:], axis=AX.X, op=OP.add)
                sr_bf = sb.tile([64, 1], bf16)
                nc.vector.tensor_copy(out=sr_bf[:], in_=sr_f[:])
                psTT = psT.tile([128, 8], f32)
                for r in range(8):
                    nc.tensor.matmul(
                        out=psTT[:, r:r + 1], lhsT=UT[:, r * 128:(r + 1) * 128],
                        rhs=sr_bf[:], start=True, stop=True)
                tdot = sb.tile([128, 8], f32)
                nc.vector.tensor_copy(out=tdot[:], in_=psTT[:])
                tw = sb.tile([128, 8], f32)
                nc.vector.tensor_tensor(out=tw[:], in0=tdot[:], in1=inv_all[:, 0:8], op=OP.mult)
                t_acc = sb.tile([128, 1], f32)
                nc.vector.tensor_reduce(out=t_acc[:], in_=tw[:], axis=AX.X, op=OP.add)

                # pair term finish: cos = dots / sqrt(uu*rr), weighted sum
                den = sb.tile([128, K], f32)
                nc.vector.tensor_tensor(out=den[:], in0=uu[:], in1=rr[:], op=OP.mult)
                lnden = sb.tile([128, K], f32)
                nc.scalar.activation(out=lnden[:], in_=den[:], func=AF.Ln)
                dinv = sb.tile([128, K], f32)
                nc.scalar.activation(out=dinv[:], in_=lnden[:], func=AF.Exp, scale=-0.5)
                cosp = sb.tile([128, K], f32)
                nc.vector.tensor_tensor(out=cosp[:], in0=dots[:], in1=dinv[:], op=OP.mult)
                cw = sb.tile([128, K], f32)
                nc.vector.tensor_tensor(out=cw[:], in0=cosp[:], in1=pw_sb[:], op=OP.mult)
                w_acc = sb.tile([128, 1], f32)
                nc.vector.tensor_reduce(out=w_acc[:], in_=cw[:], axis=AX.X, op=OP.add)

                # S2: sum rowR * ln(rowsum)
                rs_row = sb.tile([128, 8], f32)
                nc.vector.tensor_reduce(
                    out=rs_row[:], in_=rs_acc[:].rearrange("p (r g) -> p r g", g=NG),
                    axis=AX.X, op=OP.add)
                lrs = sb.tile([128, 8], f32)
                nc.scalar.activation(out=lrs[:], in_=rs_row[:], func=AF.Ln)
                s2w = sb.tile([128, 8], f32)
                nc.vector.tensor_tensor(out=s2w[:], in0=lrs[:], in1=rowr_sb[:], op=OP.mult)
                s2_acc = sb.tile([128, 1], f32)
                nc.vector.tensor_reduce(out=s2_acc[:], in_=s2w[:], axis=AX.X, op=OP.add)

                # partition-reduce the four partials via ones-matmul
                combo = sb.tile([128, 4], f32)
                nc.vector.tensor_copy(out=combo[:, 0:1], in_=s2_acc[:])
                nc.vector.tensor_copy(out=combo[:, 1:2], in_=t_acc[:])
                nc.vector.tensor_copy(out=combo[:, 2:3], in_=w_acc[:])
                nc.vector.tensor_copy(out=combo[:, 3:4], in_=m_acc[:])
                po = psT.tile([1, 4], f32)
                nc.tensor.matmul(out=po[:], lhsT=ones_f[:, 0:1], rhs=combo[:], start=True, stop=True)
                out_sb = sb.tile([1, 8], f32)
                nc.vector.memset(out_sb[:], 0.0)
                nc.vector.tensor_copy(out=out_sb[:, 0:4], in_=po[:])
                nc.sync.dma_start(out=out_d[:], in_=out_sb[:])
    nc.finalize()
    return nc


def _host_prep(inputs):
    """Dedup scatter (last write wins), shard pairs by u slab, pre-gather rows."""
    U = np.ascontiguousarray(np.asarray(inputs["user_embeddings"], dtype=np.float32))
    R = np.ascontiguousarray(np.asarray(inputs["recipe_embeddings"], dtype=np.float32))
    rat = np.asarray(inputs["ratings_scaled"], dtype=np.float32)
    css = np.asarray(inputs["cos_similarities_scaled"], dtype=np.float32)
    u = np.asarray(inputs["u_idx"]).astype(np.int64)
    i = np.asarray(inputs["i_idx"]).astype(np.int64)

    cell = u * M + i
    _, idx_rev = np.unique(cell[::-1], return_index=True)
    keep = (B - 1 - idx_rev)  # last occurrences per cell
    uu_idx = u[keep]
    ii_idx = i[keep]
    ww = (rat[keep].astype(np.float64) - FILL)

    delta = np.bincount(uu_idx, weights=ww, minlength=N)
    row_r = FILL * M + delta  # float64 [N]

    core_of = uu_idx // S
    counts = np.bincount(core_of, minlength=NCORES)
    K = max(1, int(np.ceil(counts.max() / 128)))
    cap = 128 * K

    bf = ml_dtypes.bfloat16
    in_maps = []
    bs = B // NCORES
    for c in range(NCORES):
        m = core_of == c
        n_c = int(counts[c])
        up = np.empty((cap, D), dtype=np.float32)
        rp = np.empty((cap, D), dtype=np.float32)
        pw = np.zeros(cap, dtype=np.float32)
        up[:n_c] = U[uu_idx[m]]
        rp[:n_c] = R[ii_idx[m]]
        up[n_c:] = U[0]
        rp[n_c:] = R[0]
        pw[:n_c] = ww[m]
        in_maps.append({
            "u_d": np.ascontiguousarray(U[c * S:(c + 1) * S]).reshape(128, 512),
            "r_d": R.reshape(128, 4096),
            "rowr_d": row_r[c * S:(c + 1) * S].astype(np.float32).reshape(128, 8),
            "up_d": np.ascontiguousarray(up.reshape(128, K * 64).astype(bf)),
            "rp_d": np.ascontiguousarray(rp.reshape(128, K * 64).astype(bf)),
            "pw_d": np.ascontiguousarray(pw.reshape(128, K)),
            "mse_d": np.ascontiguousarray(np.concatenate([
                rat[c * bs:(c + 1) * bs].reshape(128, 64),
                css[c * bs:(c + 1) * bs].reshape(128, 64)], axis=1)),
        })
    return in_maps, K, row_r


# column -> recipe permutation of the colsum partials ([8 groups, 1024 cols])
_c = np.arange(8192)
_RECIPE_OF_COL = (_c % 1024 % 128) * 64 + (_c // 1024) * 8 + (_c % 1024) // 128


def kernel(user_embeddings, recipe_embeddings, ratings_scaled, cos_similarities_scaled,
           u_idx, i_idx, _trace=False):
    inputs = {
        "user_embeddings": user_embeddings,
        "recipe_embeddings": recipe_embeddings,
        "ratings_scaled": ratings_scaled,
        "cos_similarities_scaled": cos_similarities_scaled,
        "u_idx": u_idx,
        "i_idx": i_idx,
    }
    in_maps, K, row_r = _host_prep(inputs)
    nc = build_nc(K)
    res = run_bass_kernel_spmd(nc, in_maps, core_ids=list(range(NCORES)), trace=_trace)
    outs = np.stack([res.results[c]["out"][0] for c in range(NCORES)]).astype(np.float64)
    cs = np.stack([res.results[c]["cs"] for c in range(NCORES)]).astype(np.float64)

    S2 = outs[:, 0].sum()
    T = outs[:, 1].sum()
    S1 = outs[:, 2].sum()
    MSE_SUM = outs[:, 3].sum()

    colsum_flat = cs.sum(axis=0).reshape(-1)  # [8192] in (group, col) order
    colsum = np.empty(M, dtype=np.float64)
    colsum[_RECIPE_OF_COL] = colsum_flat
    S3 = float(np.sum(row_r * np.log(colsum)))

    contrastive = (S2 + S3 - 2.0 * (FILL * T + S1)) / (2.0 * N)
    loss = ALPHA * contrastive + (1.0 - ALPHA) * (MSE_SUM / B)
    if _trace:
        kernel._last_results = res
    return np.float32(loss)


# revision 4
# speedup vs baseline: 2.2674x; 1.0218x over previous
"""Trainium2 Bass kernel for nn_ContrastiveMSELoss (8192x8192 cos-sim contrastive + MSE).

Sharding: 8 NeuronCores, users row-sharded 1024/core, full recipe table per core.

The loss decomposes so the 8192x8192 ratings matrix is never materialized:
    rowR[i]  = 0.1*M + sum_{final scatter cells in row i}(v - 0.1)
    S1       = sum_pairs (v-0.1)*cos[u,i]
    T        = sum_ij cos_ij = (sum_i u_i/|u_i|) . (sum_j r_j/|r_j|)
    S2       = sum_i rowR[i] * log(rowsum_exp[i])
    S3       = sum_i rowR[i] * log(colsum_exp[i])    (col_sum indexed by i: torch n==m quirk)
    loss     = 0.5*(S2 + S3 - 2*(0.1*T + S1))/(2*N) + 0.5*mean((ratings-cos_sim)^2)

ACT-bound design (~64us of exp per core is the floor):
  - r-outer main loop: per user row-tile r, 16 bf16 matmuls fill [128,2048]
    PSUM tiles; ONE wide exp per tile (amortizes ACT overhead) with accum_out
    producing rowsum partials for free. Slab r=0 runs g-wise on [128,1024]
    tiles interleaved with the recipe-transpose prelude so exp starts early.
  - Column sums: DVE linear chain colacc += ex_r on [128,4096] bf16 halves;
    per-core [128,8192] partials DMA'd out; host does the 128-way partition
    reduce + global sum + log (O(N) host work, same scale as the host bincount
    for rowR). No collective.
  - Norms via Ln+Exp(-0.5x); the activation-table registry is pinned to the
    natural_log_exp_and_others set so Exp and Ln never swap tables.
  - Pair term: host pre-gathers deduped, u-sharded pair rows as bf16; gpsimd
    (otherwise idle - no gathers, no collective) does the elementwise
    products, DVE reduces, ACT does the rsqrt via Ln+Exp.
  - Input DMAs split across the sync and scalar HWDGE queues.
"""

import sys

sys.path.insert(0, "/opt/trn_rl_repo")

import numpy as np
import ml_dtypes

import concourse.bass as bass
import concourse.bacc as bacc
import concourse.tile as tile
from concourse import mybir
from concourse.bass_utils import run_bass_kernel_spmd
from concourse.masks import make_identity

f32 = mybir.dt.float32
bf16 = mybir.dt.bfloat16
AF = mybir.ActivationFunctionType
OP = mybir.AluOpType
AX = mybir.AxisListType

NCORES = 8
N = 8192          # users
M = 8192          # recipes
D = 64
B = 65536
S = N // NCORES   # slab rows per core (1024)
NG = 8            # column groups of 1024
ALPHA = 0.5
FILL = 0.1


def _pin_act_tables():
    """Force every activation to resolve to natural_log_exp_and_others.

    The default per-instruction set choice flips between exp_and_others and
    natural_log, inserting a ~1.3us ACT_TABLE_LOAD at every Ln<->Exp
    transition. Emptying the other sets (indices preserved, so the runtime
    IDs stay correct) pins all Exp/Ln ops to the one joint set: one load.
    """
    import functools
    from concourse import hw_specs as _hw

    orig = _hw.get_activation_tables
    if getattr(orig, "_pinned_nle", False):
        return

    @functools.cache
    def pinned(arch):
        t = orig(arch)
        keep = "natural_log_exp_and_others"
        if keep not in t:
            return t
        return {name: (funcs if name == keep else set()) for name, funcs in t.items()}

    pinned._pinned_nle = True
    _hw.get_activation_tables = pinned
    bacc.get_activation_tables = pinned


_pin_act_tables()


def build_nc(K):
    """SPMD Bass program. K = pair slots per partition (128*K pairs/core)."""
    nc = bacc.Bacc(num_devices=NCORES)

    u_d = nc.declare_dram_parameter("u_d", [128, 512], f32, isOutput=False)
    r_d = nc.declare_dram_parameter("r_d", [128, 4096], f32, isOutput=False)
    rowr_d = nc.declare_dram_parameter("rowr_d", [128, 8], f32, isOutput=False)
    up_d = nc.declare_dram_parameter("up_d", [128, K * 64], bf16, isOutput=False)
    rp_d = nc.declare_dram_parameter("rp_d", [128, K * 64], bf16, isOutput=False)
    pw_d = nc.declare_dram_parameter("pw_d", [128, K], f32, isOutput=False)
    mse_d = nc.declare_dram_parameter("mse_d", [128, 128], f32, isOutput=False)
    out_d = nc.declare_dram_parameter("out", [1, 8], f32, isOutput=True)
    cs_d = nc.declare_dram_parameter("cs", [128, 8192], bf16, isOutput=True)

    with tile.TileContext(nc) as tc:
        with tc.tile_pool(name="sb", bufs=1) as sb:
            # ---- constants ----
            ident = sb.tile([128, 128], bf16)
            make_identity(nc, ident[:])
            ones_f = sb.tile([128, 1], f32)
            nc.vector.memset(ones_f[:], 1.0)

            # ---- input loads, split across sync + scalar HWDGE queues ----
            u_sb = sb.tile([128, 8, 64], f32)    # user q*8+r -> [q, r, :]
            nc.sync.dma_start(out=u_sb[:], in_=u_d[:].rearrange("p (n d) -> p n d", d=D))
            r_sb = sb.tile([128, 64, 64], f32)   # recipe p*64+n -> [p, n, :]
            for g in range(NG):
                eng = nc.sync if g < 4 else nc.scalar
                eng.dma_start(
                    out=r_sb[:, g * 8:(g + 1) * 8, :],
                    in_=r_d[:, g * 512:(g + 1) * 512].rearrange("p (n d) -> p n d", d=D))
            rowr_sb = sb.tile([128, 8], f32)
            nc.sync.dma_start(out=rowr_sb[:], in_=rowr_d[:])
            mse_sb = sb.tile([128, 128], f32)
            nc.scalar.dma_start(out=mse_sb[:], in_=mse_d[:])
            ug = sb.tile([128, K, 64], bf16)     # pair p*K+k: raw user rows
            nc.sync.dma_start(out=ug[:], in_=up_d[:].rearrange("p (k d) -> p k d", d=D))
            rg = sb.tile([128, K, 64], bf16)
            nc.scalar.dma_start(out=rg[:], in_=rp_d[:].rearrange("p (k d) -> p k d", d=D))
            pw_sb = sb.tile([128, K], f32)
            nc.sync.dma_start(out=pw_sb[:], in_=pw_d[:])

            # ---- norms: squares split gpsimd/DVE, reduces DVE, Ln+Exp ACT ----
            ssq = sb.tile([128, 72], f32)        # cols 0:8 = |u|^2, 8:72 = |r|^2
            u_bf = sb.tile([128, 8, 64], bf16)
            nc.vector.tensor_copy(out=u_bf[:], in_=u_sb[:])
            usq = sb.tile([128, 8, 64], f32)
            nc.vector.tensor_tensor(out=usq[:], in0=u_sb[:], in1=u_sb[:], op=OP.mult)
            nc.vector.tensor_reduce(out=ssq[:, 0:8], in_=usq[:], axis=AX.X, op=OP.add)
            rsqs = {}
            for g in range(NG):
                gs = slice(g * 8, (g + 1) * 8)
                rsq = sb.tile([128, 8, 64], f32, tag="rsq", bufs=4)
                eng = nc.gpsimd if g % 2 == 0 else nc.vector
                eng.tensor_tensor(out=rsq[:], in0=r_sb[:, gs, :], in1=r_sb[:, gs, :], op=OP.mult)
                nc.vector.tensor_reduce(
                    out=ssq[:, 8 + g * 8:16 + g * 8], in_=rsq[:], axis=AX.X, op=OP.add)
            lnssq = sb.tile([128, 72], f32)
            nc.scalar.activation(out=lnssq[:], in_=ssq[:], func=AF.Ln)
            inv_all = sb.tile([128, 72], f32)    # 1/sqrt(ssq): invu 0:8, invr 8:72
            nc.scalar.activation(out=inv_all[:], in_=lnssq[:], func=AF.Exp, scale=-0.5)

            # gpsimd now idle -> pair-term elementwise products
            prod = sb.tile([128, K, 64], bf16)
            usqp = sb.tile([128, K, 64], bf16)
            rsqp = sb.tile([128, K, 64], bf16)
            nc.gpsimd.tensor_tensor(out=prod[:], in0=ug[:], in1=rg[:], op=OP.mult)
            nc.gpsimd.tensor_tensor(out=usqp[:], in0=ug[:], in1=ug[:], op=OP.mult)
            nc.gpsimd.tensor_tensor(out=rsqp[:], in0=rg[:], in1=rg[:], op=OP.mult)

            UT = sb.tile([64, 1024], bf16)
            RT = sb.tile([64, 8192], bf16)       # normalized recipes, dim-major
            sr_parts = sb.tile([64, 8], f32)
            rs_acc = sb.tile([128, 36], f32)     # r0: cols 0:8 (g); r>=1: 8+(r-1)*4+gg
            colacc = sb.tile([128, 8192], bf16)
            exs = {}

            def phase_a(g, ps_pool):
                gs = slice(g * 8, (g + 1) * 8)
                rhat = sb.tile([128, 8, 64], bf16, tag="rhat", bufs=2)
                nc.vector.tensor_tensor(
                    out=rhat[:], in0=r_sb[:, gs, :],
                    in1=inv_all[:, 8 + g * 8:16 + g * 8][:, :, None].to_broadcast([128, 8, 64]),
                    op=OP.mult)
                ptr = ps_pool.tile([64, 1024], bf16, tag="tr", bufs=2)
                for t in range(8):
                    nc.tensor.transpose(
                        out=ptr[:, t * 128:(t + 1) * 128], in_=rhat[:, t, :],
                        identity=ident[:])
                nc.vector.tensor_scalar(
                    out=RT[:, g * 1024:(g + 1) * 1024], in0=ptr[:],
                    scalar1=1.0, scalar2=None, op0=OP.mult, op1=OP.add,
                    accum_out=sr_parts[:, g:g + 1])

            # ---- slab r=0, g-wise, interleaved with the transpose prelude ----
            with tc.tile_pool(name="ps1", bufs=1, space="PSUM") as ps1:
                ptu = ps1.tile([64, 1024], bf16, tag="tr", bufs=2)
                for r in range(8):
                    nc.tensor.transpose(
                        out=ptu[:, r * 128:(r + 1) * 128], in_=u_bf[:, r, :], identity=ident[:])
                nc.vector.tensor_copy(out=UT[:], in_=ptu[:])

                ex0 = sb.tile([128, 8192], bf16, tag="ex", bufs=2)
                exs[0] = ex0
                for g in range(NG):
                    phase_a(g, ps1)
                    pg = ps1.tile([128, 1024], f32, tag="cos0", bufs=2)
                    for jj in range(2):
                        nc.tensor.matmul(
                            out=pg[:, jj * 512:(jj + 1) * 512],
                            lhsT=UT[:, 0:128],
                            rhs=RT[:, g * 1024 + jj * 512:g * 1024 + (jj + 1) * 512],
                            start=True, stop=True)
                    nc.scalar.activation(
                        out=ex0[:, g * 1024:(g + 1) * 1024], in_=pg[:], func=AF.Exp,
                        scale=inv_all[:, 0:1],
                        accum_out=rs_acc[:, g:g + 1])

            # ---- slabs r=1..7 on [128,2048] tiles ----
            with tc.tile_pool(name="ps2", bufs=1, space="PSUM") as ps2:
                for r in range(1, 8):
                    ex_r = sb.tile([128, 8192], bf16, tag="ex", bufs=2)
                    exs[r] = ex_r
                    for gg in range(4):
                        pg = ps2.tile([128, 2048], f32, tag="cos", bufs=2)
                        for jj in range(4):
                            c0 = gg * 2048 + jj * 512
                            nc.tensor.matmul(
                                out=pg[:, jj * 512:(jj + 1) * 512],
                                lhsT=UT[:, r * 128:(r + 1) * 128],
                                rhs=RT[:, c0:c0 + 512],
                                start=True, stop=True)
                        nc.scalar.activation(
                            out=ex_r[:, gg * 2048:(gg + 1) * 2048], in_=pg[:], func=AF.Exp,
                            scale=inv_all[:, r:r + 1],
                            accum_out=rs_acc[:, 8 + (r - 1) * 4 + gg:9 + (r - 1) * 4 + gg])
                    # column-sum accumulate (bf16 linear chain)
                    if r == 1:
                        for h in range(2):
                            hs = slice(h * 4096, (h + 1) * 4096)
                            nc.vector.tensor_tensor(
                                out=colacc[:, hs], in0=exs[0][:, hs], in1=ex_r[:, hs], op=OP.add)
                    elif r < 7:
                        for h in range(2):
                            hs = slice(h * 4096, (h + 1) * 4096)
                            nc.vector.tensor_tensor(
                                out=colacc[:, hs], in0=colacc[:, hs], in1=ex_r[:, hs], op=OP.add)
                    else:
                        for q in range(4):
                            qs = slice(q * 2048, (q + 1) * 2048)
                            nc.vector.tensor_tensor(
                                out=colacc[:, qs], in0=colacc[:, qs], in1=ex_r[:, qs], op=OP.add)
                            eng = nc.sync if q % 2 == 0 else nc.scalar
                            eng.dma_start(out=cs_d[:, qs], in_=colacc[:, qs])
                    if r == 2:
                        md = sb.tile([128, 64], f32)
                        nc.vector.tensor_tensor(
                            out=md[:], in0=mse_sb[:, 0:64], in1=mse_sb[:, 64:128], op=OP.subtract)
                        msq = sb.tile([128, 64], f32)
                        nc.vector.tensor_tensor(out=msq[:], in0=md[:], in1=md[:], op=OP.mult)
                        m_acc = sb.tile([128, 1], f32)
                        nc.vector.tensor_reduce(out=m_acc[:], in_=msq[:], axis=AX.X, op=OP.add)
                    if r == 5:
                        dots = sb.tile([128, K], f32)
                        uu = sb.tile([128, K], f32)
                        rr = sb.tile([128, K], f32)
                        nc.vector.tensor_reduce(out=dots[:], in_=prod[:], axis=AX.X, op=OP.add)
                        nc.vector.tensor_reduce(out=uu[:], in_=usqp[:], axis=AX.X, op=OP.add)
                        nc.vector.tensor_reduce(out=rr[:], in_=rsqp[:], axis=AX.X, op=OP.add)

            # =============== tail ===============
            with tc.tile_pool(name="psT", bufs=1, space="PSUM") as psT:
                # T partial: sum_q,r invu * (u . sumRhat)
                sr_f = sb.tile([64, 1], f32)
                nc.vector.tensor_reduce(out=sr_f[:], in_=sr_parts[:], axis=AX.X, op=OP.add)
                sr_bf = sb.tile([64, 1], bf16)
                nc.vector.tensor_copy(out=sr_bf[:], in_=sr_f[:])
                psTT = psT.tile([128, 8], f32)
                for r in range(8):
                    nc.tensor.matmul(
                        out=psTT[:, r:r + 1], lhsT=UT[:, r * 128:(r + 1) * 128],
                        rhs=sr_bf[:], start=True, stop=True)
                tdot = sb.tile([128, 8], f32)
                nc.vector.tensor_copy(out=tdot[:], in_=psTT[:])
                tw = sb.tile([128, 8], f32)
                nc.vector.tensor_tensor(out=tw[:], in0=tdot[:], in1=inv_all[:, 0:8], op=OP.mult)
                t_acc = sb.tile([128, 1], f32)
                nc.vector.tensor_reduce(out=t_acc[:], in_=tw[:], axis=AX.X, op=OP.add)

                # S2: sum rowR * ln(rowsum)
                rs_row = sb.tile([128, 8], f32)
                nc.vector.tensor_reduce(
                    out=rs_row[:, 0:1], in_=rs_acc[:, 0:8], axis=AX.X, op=OP.add)
                for r in range(1, 8):
                    nc.vector.tensor_reduce(
                        out=rs_row[:, r:r + 1], in_=rs_acc[:, 8 + (r - 1) * 4:8 + r * 4],
                        axis=AX.X, op=OP.add)
                lrs = sb.tile([128, 8], f32)
                nc.scalar.activation(out=lrs[:], in_=rs_row[:], func=AF.Ln)

                # pair term finish: cos = dots / sqrt(uu*rr), weighted sum
                den = sb.tile([128, K], f32)
                nc.vector.tensor_tensor(out=den[:], in0=uu[:], in1=rr[:], op=OP.mult)
                lnden = sb.tile([128, K], f32)
                nc.scalar.activation(out=lnden[:], in_=den[:], func=AF.Ln)
                dinv = sb.tile([128, K], f32)
                nc.scalar.activation(out=dinv[:], in_=lnden[:], func=AF.Exp, scale=-0.5)
                cosp = sb.tile([128, K], f32)
                nc.vector.tensor_tensor(out=cosp[:], in0=dots[:], in1=dinv[:], op=OP.mult)
                cw = sb.tile([128, K], f32)
                nc.vector.tensor_tensor(out=cw[:], in0=cosp[:], in1=pw_sb[:], op=OP.mult)
                w_acc = sb.tile([128, 1], f32)
                nc.vector.tensor_reduce(out=w_acc[:], in_=cw[:], axis=AX.X, op=OP.add)

                s2w = sb.tile([128, 8], f32)
                nc.vector.tensor_tensor(out=s2w[:], in0=lrs[:], in1=rowr_sb[:], op=OP.mult)
                s2_acc = sb.tile([128, 1], f32)
                nc.vector.tensor_reduce(out=s2_acc[:], in_=s2w[:], axis=AX.X, op=OP.add)

                # partition-reduce the four partials via ones-matmul
                combo = sb.tile([128, 4], f32)
                nc.vector.tensor_copy(out=combo[:, 0:1], in_=s2_acc[:])
                nc.vector.tensor_copy(out=combo[:, 1:2], in_=t_acc[:])
                nc.vector.tensor_copy(out=combo[:, 2:3], in_=w_acc[:])
                nc.vector.tensor_copy(out=combo[:, 3:4], in_=m_acc[:])
                po = psT.tile([1, 4], f32)
                nc.tensor.matmul(out=po[:], lhsT=ones_f[:, 0:1], rhs=combo[:], start=True, stop=True)
                out_sb = sb.tile([1, 8], f32)
                nc.vector.memset(out_sb[:], 0.0)
                nc.vector.tensor_copy(out=out_sb[:, 0:4], in_=po[:])
                nc.sync.dma_start(out=out_d[:], in_=out_sb[:])
    nc.finalize()
    return nc


def _host_prep(inputs):
    """Dedup scatter (last write wins), shard pairs by u slab, pre-gather rows."""
    U = np.ascontiguousarray(np.asarray(inputs["user_embeddings"], dtype=np.float32))
    R = np.ascontiguousarray(np.asarray(inputs["recipe_embeddings"], dtype=np.float32))
    rat = np.asarray(inputs["ratings_scaled"], dtype=np.float32)
    css = np.asarray(inputs["cos_similarities_scaled"], dtype=np.float32)
    u = np.asarray(inputs["u_idx"]).astype(np.int64)
    i = np.asarray(inputs["i_idx"]).astype(np.int64)

    cell = u * M + i
    _, idx_rev = np.unique(cell[::-1], return_index=True)
    keep = (B - 1 - idx_rev)  # last occurrences per cell
    uu_idx = u[keep]
    ii_idx = i[keep]
    ww = (rat[keep].astype(np.float64) - FILL)

    delta = np.bincount(uu_idx, weights=ww, minlength=N)
    row_r = FILL * M + delta  # float64 [N]

    core_of = uu_idx // S
    counts = np.bincount(core_of, minlength=NCORES)
    K = max(1, int(np.ceil(counts.max() / 128)))
    cap = 128 * K

    bf = ml_dtypes.bfloat16
    in_maps = []
    bs = B // NCORES
    for c in range(NCORES):
        m = core_of == c
        n_c = int(counts[c])
        up = np.empty((cap, D), dtype=np.float32)
        rp = np.empty((cap, D), dtype=np.float32)
        pw = np.zeros(cap, dtype=np.float32)
        up[:n_c] = U[uu_idx[m]]
        rp[:n_c] = R[ii_idx[m]]
        up[n_c:] = U[0]
        rp[n_c:] = R[0]
        pw[:n_c] = ww[m]
        in_maps.append({
            "u_d": np.ascontiguousarray(U[c * S:(c + 1) * S]).reshape(128, 512),
            "r_d": R.reshape(128, 4096),
            "rowr_d": row_r[c * S:(c + 1) * S].astype(np.float32).reshape(128, 8),
            "up_d": np.ascontiguousarray(up.reshape(128, K * 64).astype(bf)),
            "rp_d": np.ascontiguousarray(rp.reshape(128, K * 64).astype(bf)),
            "pw_d": np.ascontiguousarray(pw.reshape(128, K)),
            "mse_d": np.ascontiguousarray(np.concatenate([
                rat[c * bs:(c + 1) * bs].reshape(128, 64),
                css[c * bs:(c + 1) * bs].reshape(128, 64)], axis=1)),
        })
    return in_maps, K, row_r


# column -> recipe permutation of the colsum partials (col = g*1024 + t*128 + p)
_c = np.arange(8192)
_RECIPE_OF_COL = (_c % 1024 % 128) * 64 + (_c // 1024) * 8 + (_c % 1024) // 128


def kernel(user_embeddings, recipe_embeddings, ratings_scaled, cos_similarities_scaled,
           u_idx, i_idx, _trace=False):
    inputs = {
        "user_embeddings": user_embeddings,
        "recipe_embeddings": recipe_embeddings,
        "ratings_scaled": ratings_scaled,
        "cos_similarities_scaled": cos_similarities_scaled,
        "u_idx": u_idx,
        "i_idx": i_idx,
    }
    in_maps, K, row_r = _host_prep(inputs)
    nc = build_nc(K)
    res = run_bass_kernel_spmd(nc, in_maps, core_ids=list(range(NCORES)), trace=_trace)
    outs = np.stack([res.results[c]["out"][0] for c in range(NCORES)]).astype(np.float64)
    cs = np.stack([res.results[c]["cs"] for c in range(NCORES)]).astype(np.float64)

    S2 = outs[:, 0].sum()
    T = outs[:, 1].sum()
    S1 = outs[:, 2].sum()
    MSE_SUM = outs[:, 3].sum()

    colsum_flat = cs.sum(axis=(0, 1))  # [8192] in column order
    colsum = np.empty(M, dtype=np.float64)
    colsum[_RECIPE_OF_COL] = colsum_flat
    S3 = float(np.sum(row_r * np.log(colsum)))

    contrastive = (S2 + S3 - 2.0 * (FILL * T + S1)) / (2.0 * N)
    loss = ALPHA * contrastive + (1.0 - ALPHA) * (MSE_SUM / B)
    if _trace:
        kernel._last_results = res
    return np.float32(loss)
